# Initial kernel scaffold
#
"""Trainium2 Bass kernel: multi-head attention with time-decay bias + unify + layernorm.

Returns (layernorm_out [4,2048,512], attn [4,8,2048,2048]) like the reference.

Sharding: 8 cores; core c handles batch c//2, query-half c%2 (1024 queries,
all 8 heads). Fully independent SPMD, no collectives; host concatenates.

Per-core device algorithm:
  qkv phase : qT/kT computed transposed ([inner, n]) via lhsT=w_qkv chunks,
              rhs=xT; v computed natural ([n, dh]) with a ones column
              appended per head (for row-sum extraction).
  pass 1    : ST[keys, q] = kT.T @ qT per head (head-PAIR row-packed K=64
              matmuls), PT = exp(ST + decay) on ACT (decay = per-partition
              bias).  attn@v:  outT[65, q] += [v|1].T @ PT  accumulated over
              key chunks; row 64 = softmax denominator r[q].
  pass 2    : S[q, keys] = q_aug.T @ k_aug (65-dim contraction: 65th
              component carries the decay bias), then
              attn = exp(S + ln(1/r)) on ACT -> DMA out (normalized, fp32).
  unify+LN  : attn-out rows normalized by 1/r (PE broadcast of 1/r along
              partitions), stacked transposed -> lhsT for unify matmul,
              + bias, layernorm, DMA out.

All matmuls run in float32r (fp32 storage, full PE rate at N>=512).
"""

import numpy as np

B, N, D = 4, 2048, 512
H, DH = 8, 64
INNER = H * DH  # 512
NQ = N // 2     # queries per core
NCORES = 8
KT = N // 128   # 16 key chunks
QT = NQ // 128  # 8 query tiles per core
PAIRS = H // 2  # 4 head pairs
DECAY_TAU = 50.0
LN_EPS = 1e-5

_PROG = None


def _build_program():
    import concourse.bass as bass
    import concourse.mybir as mybir
    import concourse.tile as tile

    f32 = mybir.dt.float32
    f32r = mybir.dt.float32r
    AF = mybir.ActivationFunctionType
    OP = mybir.AluOpType

    def r(ap):
        return ap.bitcast(f32r)

    nc = bass.Bass()

    xT_d = nc.dram_tensor("xT", [D, N], f32, kind="ExternalInput").ap()
    xTq_d = nc.dram_tensor("xTq", [D, NQ], f32, kind="ExternalInput").ap()
    wqkv_d = nc.dram_tensor("wqkv", [D, 3 * INNER], f32, kind="ExternalInput").ap()
    wu_d = nc.dram_tensor("wu", [INNER, D], f32, kind="ExternalInput").ap()
    bu_d = nc.dram_tensor("bu", [1, D], f32, kind="ExternalInput").ap()
    gam_d = nc.dram_tensor("gamma", [1, D], f32, kind="ExternalInput").ap()
    bet_d = nc.dram_tensor("beta", [1, D], f32, kind="ExternalInput").ap()
    dkr_d = nc.dram_tensor("decay_row", [1, N], f32, kind="ExternalInput").ap()
    dkp_d = nc.dram_tensor("decay_pc", [128, KT], f32, kind="ExternalInput").ap()

    attn_d = nc.dram_tensor("attn", [H, NQ, N], f32, kind="ExternalOutput").ap()
    y_d = nc.dram_tensor("y", [NQ, D], f32, kind="ExternalOutput").ap()

    with tile.TileContext(nc) as tc:
        with (
            tc.tile_pool(name="consts", bufs=1) as cp,
            tc.tile_pool(name="xin", bufs=1) as xp,
            tc.tile_pool(name="wts", bufs=1) as wp,
            tc.tile_pool(name="qk", bufs=1) as qkp,
            tc.tile_pool(name="vp", bufs=1) as vp,
            tc.tile_pool(name="aug", bufs=1) as augp,
            tc.tile_pool(name="pt", bufs=2) as ptp,
            tc.tile_pool(name="aout", bufs=3) as aoutp,
            tc.tile_pool(name="aflat", bufs=1) as afp,
            tc.tile_pool(name="small", bufs=8) as smallp,
            tc.tile_pool(name="yout", bufs=2) as yp,
            tc.tile_pool(name="ps_qkv", bufs=2, space="PSUM") as ps_qkv,
            tc.tile_pool(name="ps_st", bufs=1, space="PSUM") as ps_st,
            tc.tile_pool(name="ps_acc", bufs=2, space="PSUM") as ps_acc,
            tc.tile_pool(name="ps_bc", bufs=2, space="PSUM") as ps_bc,
            tc.tile_pool(name="ps_s", bufs=2, space="PSUM") as ps_s,
            tc.tile_pool(name="ps_u", bufs=2, space="PSUM") as ps_u,
        ):
            # ---------- constant / weight loads ----------
            xT_sb = xp.tile([128, 4, N], f32)
            xTq_sb = xp.tile([128, 4, NQ], f32)
            w_sb = wp.tile([128, 4, 3 * INNER], f32)
            wu_sb = wp.tile([128, 4, D], f32)
            for dc in range(4):
                nc.sync.dma_start(
                    out=xT_sb[:, dc, :], in_=xT_d.rearrange("(c p) n -> c p n", p=128)[dc]
                )
                nc.sync.dma_start(
                    out=xTq_sb[:, dc, :],
                    in_=xTq_d.rearrange("(c p) n -> c p n", p=128)[dc],
                )
                nc.sync.dma_start(
                    out=w_sb[:, dc, :], in_=wqkv_d.rearrange("(c p) n -> c p n", p=128)[dc]
                )
                nc.sync.dma_start(
                    out=wu_sb[:, dc, :], in_=wu_d.rearrange("(c p) n -> c p n", p=128)[dc]
                )
            dkp_sb = cp.tile([128, KT], f32)
            nc.sync.dma_start(out=dkp_sb, in_=dkp_d)

            ones_col = cp.tile([1, 128], f32)
            nc.vector.memset(ones_col, 1.0)
            ones_1 = cp.tile([1, 1], f32)
            nc.vector.memset(ones_1, 1.0)
            eps_sb = cp.tile([128, 1], f32)
            nc.vector.memset(eps_sb, LN_EPS)

            # broadcast b_unify / gamma / beta along partitions via K=1 matmul
            def bcast_row(dram_row):
                row = smallp.tile([1, D], f32, tag="vecrow")
                nc.sync.dma_start(out=row, in_=dram_row)
                ps = ps_bc.tile([128, D], f32, tag="vecbc")
                nc.tensor.matmul(ps, r(ones_col), r(row), start=True, stop=True)
                sb = cp.tile([128, D], f32, tag=f"bc{dram_row.tensor.name}")
                nc.vector.tensor_copy(sb, ps)
                return sb

            bu_bc = bcast_row(bu_d)
            gam_bc = bcast_row(gam_d)
            bet_bc = bcast_row(bet_d)

            # ---------- qkv phase ----------
            qT_sb = qkp.tile([128, 4, NQ], f32)   # q/8, transposed, pair-stacked
            kT_sb = qkp.tile([128, 4, N], f32)    # k, transposed, pair-stacked
            v_sb = vp.tile([128, KT, H, DH + 1], f32)  # v natural + ones col
            nc.vector.memset(v_sb[:, :, :, DH : DH + 1], 1.0)

            for mi in range(4):  # q inner chunks (cols mi*128..)
                ps = ps_qkv.tile([128, NQ], f32, tag="q")
                for dc in range(4):
                    for ns in range(NQ // 512):
                        nc.tensor.matmul(
                            ps[:, ns * 512 : (ns + 1) * 512],
                            r(w_sb[:, dc, mi * 128 : (mi + 1) * 128]),
                            r(xTq_sb[:, dc, ns * 512 : (ns + 1) * 512]),
                            start=(dc == 0),
                            stop=(dc == 3),
                        )
                nc.vector.tensor_copy(qT_sb[:, mi, :], ps)

            for mi in range(4):  # k inner chunks (cols 512+mi*128..)
                ps = ps_qkv.tile([128, N], f32, tag="k")
                for dc in range(4):
                    for ns in range(N // 512):
                        nc.tensor.matmul(
                            ps[:, ns * 512 : (ns + 1) * 512],
                            r(w_sb[:, dc, INNER + mi * 128 : INNER + (mi + 1) * 128]),
                            r(xT_sb[:, dc, ns * 512 : (ns + 1) * 512]),
                            start=(dc == 0),
                            stop=(dc == 3),
                        )
                nc.vector.tensor_copy(kT_sb[:, mi, :], ps)

            for ni in range(KT):  # v natural, row chunks
                ps = ps_qkv.tile([128, D], f32, tag="v")
                for dc in range(4):
                    nc.tensor.matmul(
                        ps,
                        r(xT_sb[:, dc, ni * 128 : (ni + 1) * 128]),
                        r(w_sb[:, dc, 2 * INNER : 3 * INNER]),
                        start=(dc == 0),
                        stop=(dc == 3),
                    )
                nc.vector.tensor_copy(
                    v_sb[:, ni, :, 0:DH], ps.rearrange("p (h x) -> p h x", h=H)
                )

            # aug tiles for pass 2 (rebuilt per head)
            qaug = augp.tile([DH + 1, NQ], f32)
            kaug = augp.tile([DH + 1, N], f32)
            nc.vector.memset(qaug[DH : DH + 1, :], 1.0)
            nc.sync.dma_start(out=kaug[DH : DH + 1, :], in_=dkr_d)

            aflat = afp.tile([128, 4, NQ], f32)  # normalized attn-out, transposed

            # ---------- attention: per head pair ----------
            for pair in range(PAIRS):
                outT = [
                    ps_acc.tile([DH + 1, NQ], f32, tag="outT") for _ in range(2)
                ]
                # pass 1 + attn@v
                for kt in range(KT):
                    st2 = ps_st.tile([128, 2 * NQ], f32, tag="st2")
                    for hi in range(2):
                        bp = hi * 64
                        for ns in range(NQ // 512):
                            nc.tensor.matmul(
                                st2[:, hi * NQ + ns * 512 : hi * NQ + (ns + 1) * 512],
                                r(kT_sb[bp : bp + 64, pair, kt * 128 : (kt + 1) * 128]),
                                r(qT_sb[bp : bp + 64, pair, ns * 512 : (ns + 1) * 512]),
                                start=True,
                                stop=True,
                                tile_position=(bp, 0),
                            )
                    pt = ptp.tile([128, 2 * NQ], f32)
                    nc.scalar.activation(
                        pt, st2, AF.Exp, bias=dkp_sb[:, kt : kt + 1], scale=1.0
                    )
                    for hi in range(2):
                        h = pair * 2 + hi
                        for ns in range(NQ // 512):
                            nc.tensor.matmul(
                                outT[hi][:, ns * 512 : (ns + 1) * 512],
                                r(v_sb[:, kt, h, :]),
                                r(pt[:, hi * NQ + ns * 512 : hi * NQ + (ns + 1) * 512]),
                                start=(kt == 0),
                                stop=(kt == KT - 1),
                                skip_group_check=True,
                            )

                # tail: r -> 1/r, normalize attn-out into aflat
                rinv = [None, None]
                for hi in range(2):
                    rv = smallp.tile([1, NQ], f32, tag="rinv")
                    nc.vector.reciprocal(rv, outT[hi][DH : DH + 1, :])
                    rinv[hi] = rv
                    bc = ps_bc.tile([128, NQ], f32, tag="rbc")
                    for ns in range(NQ // 512):
                        nc.tensor.matmul(
                            bc[:, ns * 512 : (ns + 1) * 512],
                            r(ones_col),
                            r(rv[:, ns * 512 : (ns + 1) * 512]),
                            start=True,
                            stop=True,
                        )
                    ot = smallp.tile([DH, NQ], f32, tag="otsb")
                    nc.vector.tensor_copy(ot, outT[hi][0:DH, :])
                    nc.vector.tensor_mul(
                        aflat[hi * 64 : hi * 64 + DH, pair, :], ot, bc[0:DH, :]
                    )

                # pass 2: normalized attn output, per head
                for hi in range(2):
                    h = pair * 2 + hi
                    bp = hi * 64
                    nc.gpsimd.tensor_copy(
                        kaug[0:DH, :], kT_sb[bp : bp + DH, pair, :]
                    )
                    nc.gpsimd.tensor_copy(
                        qaug[0:DH, :], qT_sb[bp : bp + DH, pair, :]
                    )
                    # ln(1/r) into q-partition orientation via K=1 matmuls
                    rp = ps_s.tile([128, QT], f32, tag="rp")
                    for qt in range(QT):
                        nc.tensor.matmul(
                            rp[:, qt : qt + 1],
                            r(rinv[hi][:, qt * 128 : (qt + 1) * 128]),
                            r(ones_1),
                            start=True,
                            stop=True,
                        )
                    lnri = smallp.tile([128, QT], f32, tag="lnri")
                    nc.scalar.activation(lnri, rp, AF.Ln)
                    for qt in range(QT):
                        s = ps_s.tile([128, N], f32, tag="s")
                        for ns in range(N // 512):
                            nc.tensor.matmul(
                                s[:, ns * 512 : (ns + 1) * 512],
                                r(qaug[:, qt * 128 : (qt + 1) * 128]),
                                r(kaug[:, ns * 512 : (ns + 1) * 512]),
                                start=True,
                                stop=True,
                            )
                        at = aoutp.tile([128, N], f32)
                        nc.scalar.activation(
                            at, s, AF.Exp, bias=lnri[:, qt : qt + 1], scale=1.0
                        )
                        nc.sync.dma_start(
                            out=attn_d[h, qt * 128 : (qt + 1) * 128, :], in_=at
                        )

            # ---------- unify + layernorm ----------
            for rt in range(QT):
                psu = ps_u.tile([128, D], f32, tag="u")
                for ic in range(4):
                    nc.tensor.matmul(
                        psu,
                        r(aflat[:, ic, rt * 128 : (rt + 1) * 128]),
                        r(wu_sb[:, ic, :]),
                        start=(ic == 0),
                        stop=(ic == 3),
                    )
                tmp = yp.tile([128, D], f32, tag="tmp")
                nc.vector.tensor_add(tmp, psu, bu_bc)
                stats = smallp.tile([128, 6], f32, tag="stats")
                nc.vector.bn_stats(stats, tmp)
                mv = smallp.tile([128, 2], f32, tag="mv")
                nc.vector.bn_aggr(mv, stats)
                sq = smallp.tile([128, 1], f32, tag="sq")
                nc.scalar.activation(sq, mv[:, 1:2], AF.Sqrt, bias=eps_sb)
                rstd = smallp.tile([128, 1], f32, tag="rstd")
                nc.vector.reciprocal(rstd, sq)
                xn = yp.tile([128, D], f32, tag="xn")
                nc.vector.tensor_scalar(
                    out=xn,
                    in0=tmp,
                    scalar1=mv[:, 0:1],
                    scalar2=rstd,
                    op0=OP.subtract,
                    op1=OP.mult,
                )
                nc.vector.tensor_mul(xn, xn, gam_bc)
                nc.vector.tensor_add(xn, xn, bet_bc)
                nc.sync.dma_start(out=y_d[rt * 128 : (rt + 1) * 128, :], in_=xn)

    return nc


def _make_in_maps(x, w_qkv, w_unify, b_unify, ln_gamma, ln_beta):
    wq = np.array(w_qkv, dtype=np.float32, copy=True)
    wq[:, :INNER] *= DH ** -0.5
    wq = np.ascontiguousarray(wq)
    wu = np.ascontiguousarray(w_unify.astype(np.float32))
    decay = np.exp(-np.arange(N, dtype=np.float64) / DECAY_TAU).astype(np.float32)
    decay_row = np.ascontiguousarray(decay.reshape(1, N))
    decay_pc = np.ascontiguousarray(decay.reshape(KT, 128).T)
    bu = np.ascontiguousarray(b_unify.astype(np.float32).reshape(1, D))
    gam = np.ascontiguousarray(ln_gamma.astype(np.float32).reshape(1, D))
    bet = np.ascontiguousarray(ln_beta.astype(np.float32).reshape(1, D))

    in_maps = []
    for c in range(NCORES):
        b_i, qh = c // 2, c % 2
        xT = np.ascontiguousarray(x[b_i].T.astype(np.float32))
        xTq = np.ascontiguousarray(xT[:, qh * NQ : (qh + 1) * NQ])
        in_maps.append(
            {
                "xT": xT,
                "xTq": xTq,
                "wqkv": wq,
                "wu": wu,
                "bu": bu,
                "gamma": gam,
                "beta": bet,
                "decay_row": decay_row,
                "decay_pc": decay_pc,
            }
        )
    return in_maps


def kernel(x, w_qkv, w_unify, b_unify, ln_gamma, ln_beta, _collect_results=None):
    from concourse.bass_utils import run_bass_kernel_spmd

    global _PROG
    if _PROG is None:
        _PROG = _build_program()
    nc = _PROG

    in_maps = _make_in_maps(x, w_qkv, w_unify, b_unify, ln_gamma, ln_beta)
    res = run_bass_kernel_spmd(nc, in_maps, list(range(NCORES)))
    if _collect_results is not None:
        _collect_results.append(res)
    outs = res.results

    y = np.empty((B, N, D), np.float32)
    attn = np.empty((B, H, N, N), np.float32)
    for c in range(NCORES):
        b_i, qh = c // 2, c % 2
        sl = slice(qh * NQ, (qh + 1) * NQ)
        y[b_i, sl] = outs[c]["y"]
        attn[b_i, :, sl] = outs[c]["attn"]
    return y, attn


if __name__ == "__main__":
    nc = _build_program()
    print("program built OK; instructions:", len(nc.m.functions[0].instructions))


# revision 14
# speedup vs baseline: 1.1807x; 1.1807x over previous
"""Trainium2 Bass kernel: multi-head attention with time-decay bias + unify + layernorm.

Returns (layernorm_out [4,2048,512], attn [4,8,2048,2048]) like the reference.

Sharding: 8 cores; core c handles batch c//2, query-half c%2 (1024 queries,
all 8 heads). Fully independent SPMD, no collectives; host concatenates.

Per-core device algorithm:
  qkv phase : qT/kT computed transposed ([inner, n]) via lhsT=w_qkv chunks,
              rhs=xT; v computed natural ([n, dh]) with a ones column
              appended per head (for row-sum extraction).
  pass 1    : ST[keys, q] = kT.T @ qT per head (head-PAIR row-packed K=64
              matmuls), PT = exp(ST + decay) on ACT (decay = per-partition
              bias).  attn@v:  outT[65, q] += [v|1].T @ PT  accumulated over
              key chunks; row 64 = softmax denominator r[q].
  pass 2    : S[q, keys] = q_aug.T @ k_aug (65-dim contraction: 65th
              component carries the decay bias), then
              attn = exp(S + ln(1/r)) on ACT -> DMA out (normalized, fp32).
  unify+LN  : attn-out rows normalized by 1/r (PE broadcast of 1/r along
              partitions), stacked transposed -> lhsT for unify matmul,
              + bias, layernorm, DMA out.

Score-path matmuls run in bf16 (fp32 PSUM accumulation); the unify matmul
runs in float32r.  One global PSUM pool (all tiles <= 2 banks) avoids
pool-boundary drains.
"""

import numpy as np

B, N, D = 4, 2048, 512
H, DH = 8, 64
INNER = H * DH  # 512
NQ = N // 2     # queries per core
NCORES = 8
KT = N // 128   # 16 key chunks
QT = NQ // 128  # 8 query tiles per core
PAIRS = H // 2  # 4 head pairs
DECAY_TAU = 50.0
LN_EPS = 1e-5

_PROG = None


def _build_program():
    from contextlib import ExitStack

    import concourse.bass as bass
    import concourse.mybir as mybir
    import concourse.tile as tile
    from concourse import bacc

    f32 = mybir.dt.float32
    f32r = mybir.dt.float32r
    bf16 = mybir.dt.bfloat16
    AF = mybir.ActivationFunctionType
    OP = mybir.AluOpType

    nc = bacc.Bacc("TRN2", target_bir_lowering=False, debug=False)

    xT_d = nc.dram_tensor("xT", [D, N], bf16, kind="ExternalInput").ap()
    xTq_d = nc.dram_tensor("xTq", [D, NQ], bf16, kind="ExternalInput").ap()
    wqkv_d = nc.dram_tensor("wqkv", [D, 3 * INNER], bf16, kind="ExternalInput").ap()
    wu_d = nc.dram_tensor("wu", [INNER, D], f32r, kind="ExternalInput").ap()
    bu_d = nc.dram_tensor("bu", [1, D], f32r, kind="ExternalInput").ap()
    gam_d = nc.dram_tensor("gamma", [1, D], f32r, kind="ExternalInput").ap()
    bet_d = nc.dram_tensor("beta", [1, D], f32r, kind="ExternalInput").ap()
    ones_d = nc.dram_tensor("ones_row", [1, NQ], f32r, kind="ExternalInput").ap()
    onesb_d = nc.dram_tensor("ones_bf", [1, NQ], bf16, kind="ExternalInput").ap()
    dkb_d = nc.dram_tensor("decay_bf", [1, N], bf16, kind="ExternalInput").ap()
    dkp_d = nc.dram_tensor("decay_pc", [128, KT], f32, kind="ExternalInput").ap()

    attn_d = nc.dram_tensor("attn", [H, NQ, N], f32, kind="ExternalOutput").ap()
    y_d = nc.dram_tensor("y", [NQ, D], f32, kind="ExternalOutput").ap()

    with tile.TileContext(nc) as tc, ExitStack() as ctx:
        ep = ctx.enter_context
        cp = ep(tc.tile_pool(name="consts", bufs=1))
        wp = ep(tc.tile_pool(name="wts", bufs=1))
        qkp = ep(tc.tile_pool(name="qk", bufs=1))
        vp = ep(tc.tile_pool(name="vp", bufs=1))
        afp = ep(tc.tile_pool(name="aflat", bufs=1))
        rinvp = ep(tc.tile_pool(name="rinvp", bufs=2))
        smallp = ep(tc.tile_pool(name="small", bufs=2))
        yp = ep(tc.tile_pool(name="yout", bufs=2))
        # one PSUM pool for the whole kernel: tags "a" and "b", tiles <= 2 banks
        psp = ep(tc.tile_pool(name="ps", bufs=2, space="PSUM"))

        # ---- weights / constants (single-shot DMAs) ----
        wu_sb = wp.tile([128, 4, D], f32r)
        nc.sync.dma_start(out=wu_sb, in_=wu_d.rearrange("(c p) n -> p c n", p=128))
        dkp_sb = cp.tile([128, KT], f32)
        nc.sync.dma_start(out=dkp_sb, in_=dkp_d)
        ones_col = cp.tile([1, 128], f32r)
        nc.sync.dma_start(out=ones_col, in_=ones_d[:, 0:128])
        eps_sb = cp.tile([128, 1], f32)
        nc.vector.memset(eps_sb, LN_EPS)
        ones_f1 = cp.tile([1, 1], f32)
        nc.vector.memset(ones_f1, 1.0)

        # broadcast b_unify / gamma / beta along partitions via K=1 matmul
        def bcast_row(dram_row, nm):
            row = smallp.tile([1, D], f32r, tag="vecrow", name=f"row_{nm}")
            nc.sync.dma_start(out=row, in_=dram_row)
            ps = psp.tile([128, D], f32, tag="a", name=f"bc_{nm}")
            nc.tensor.matmul(ps, ones_col, row, start=True, stop=True)
            sb = cp.tile([128, D], f32, tag=f"bcs_{nm}", name=f"bcs_{nm}")
            nc.vector.tensor_copy(sb, ps)
            return sb

        bu_bc = bcast_row(bu_d, "bu")
        gam_bc = bcast_row(gam_d, "gam")
        bet_bc = bcast_row(bet_d, "bet")

        qT_sb = qkp.tile([128, 4, NQ], bf16)   # q/8, transposed, pair-stacked
        kT_sb = qkp.tile([128, 4, N], bf16)    # k, transposed, pair-stacked
        v_sb = vp.tile([128, KT, H, DH + 1], bf16)  # v natural + ones col
        ones_bc = bass.AP(
            tensor=onesb_d.tensor, offset=0, ap=[[0, 128], [1, KT * H]]
        )
        nc.sync.dma_start(out=v_sb[:, :, :, DH], in_=ones_bc)
        aflat = afp.tile([128, 4, NQ], f32r)   # normalized attn-out, transposed
        # persistent per-head augmented q/k (65th row: ones / decay) for pass 2
        qaug = afp.tile([DH + 1, H, NQ], bf16)
        kaug = afp.tile([DH + 1, H, N], bf16)
        qones_bc = bass.AP(
            tensor=onesb_d.tensor, offset=0, ap=[[0, 1], [0, H], [1, NQ]]
        )
        nc.sync.dma_start(out=qaug[DH : DH + 1, :, :], in_=qones_bc)
        kdec_bc = bass.AP(
            tensor=dkb_d.tensor, offset=0, ap=[[0, 1], [0, H], [1, N]]
        )
        nc.sync.dma_start(out=kaug[DH : DH + 1, :, :], in_=kdec_bc)

        # ---------- phase 1: qkv ----------
        with (
            tc.tile_pool(name="xin", bufs=1) as xp,
            tc.tile_pool(name="win", bufs=1) as wqp,
        ):
            xT_sb = xp.tile([128, 4, N], bf16)
            xTq_sb = xp.tile([128, 4, NQ], bf16)
            w_sb = wqp.tile([128, 4, 3 * INNER], bf16)
            nc.sync.dma_start(
                out=xT_sb, in_=xT_d.rearrange("(c p) n -> p c n", p=128)
            )
            nc.sync.dma_start(
                out=xTq_sb, in_=xTq_d.rearrange("(c p) n -> p c n", p=128)
            )
            nc.sync.dma_start(
                out=w_sb, in_=wqkv_d.rearrange("(c p) n -> p c n", p=128)
            )

            for mi in range(4):  # q inner chunks
                ps = psp.tile([128, NQ], f32, tag="a", name=f"psq{mi}")
                for dc in range(4):
                    for ns in range(NQ // 512):
                        nc.tensor.matmul(
                            ps[:, ns * 512 : (ns + 1) * 512],
                            w_sb[:, dc, mi * 128 : (mi + 1) * 128],
                            xTq_sb[:, dc, ns * 512 : (ns + 1) * 512],
                            start=(dc == 0),
                            stop=(dc == 3),
                        )
                nc.vector.tensor_copy(qT_sb[:, mi, :], ps)
                for hi in range(2):
                    nc.vector.tensor_copy(
                        qaug[0:DH, mi * 2 + hi, :], ps[hi * 64 : hi * 64 + DH, :]
                    )

            for mi in range(4):  # k inner chunks
                for half in range(2):
                    ps = psp.tile([128, NQ], f32, tag="b", name=f"psk{mi}_{half}")
                    for dc in range(4):
                        for ns in range(NQ // 512):
                            nc.tensor.matmul(
                                ps[:, ns * 512 : (ns + 1) * 512],
                                w_sb[:, dc, INNER + mi * 128 : INNER + (mi + 1) * 128],
                                xT_sb[
                                    :,
                                    dc,
                                    half * NQ + ns * 512 : half * NQ + (ns + 1) * 512,
                                ],
                                start=(dc == 0),
                                stop=(dc == 3),
                            )
                    nc.vector.tensor_copy(
                        kT_sb[:, mi, half * NQ : (half + 1) * NQ], ps
                    )
                    for hi in range(2):
                        nc.vector.tensor_copy(
                            kaug[0:DH, mi * 2 + hi, half * NQ : (half + 1) * NQ],
                            ps[hi * 64 : hi * 64 + DH, :],
                        )

            for ni in range(KT):  # v natural, row chunks
                ps = psp.tile([128, D], f32, tag="a", name=f"psv{ni}")
                for dc in range(4):
                    nc.tensor.matmul(
                        ps,
                        xT_sb[:, dc, ni * 128 : (ni + 1) * 128],
                        w_sb[:, dc, 2 * INNER : 3 * INNER],
                        start=(dc == 0),
                        stop=(dc == 3),
                    )
                nc.vector.tensor_copy(
                    v_sb[:, ni, :, 0:DH], ps.rearrange("p (h x) -> p h x", h=H)
                )

        # ---------- attention ----------
        with (
            tc.tile_pool(name="pt", bufs=3) as ptp,
            tc.tile_pool(name="aout", bufs=4) as aoutp,
            tc.tile_pool(name="otsb", bufs=2) as otp,
        ):
            for pair in range(PAIRS):
                # ---- pass 1 + attn@v ----
                outT = [
                    psp.tile([DH + 1, NQ], f32, tag="b", name=f"outT{pair}_{i}")
                    for i in range(2)
                ]
                for kt in range(KT):
                    for hi in range(2):
                        h = pair * 2 + hi
                        bp = hi * 64
                        st = psp.tile(
                            [128, NQ], f32, tag="a", name=f"st{pair}_{kt}_{hi}"
                        )
                        for ns in range(NQ // 512):
                            nc.tensor.matmul(
                                st[:, ns * 512 : (ns + 1) * 512],
                                kT_sb[bp : bp + 64, pair, kt * 128 : (kt + 1) * 128],
                                qT_sb[bp : bp + 64, pair, ns * 512 : (ns + 1) * 512],
                                start=True,
                                stop=True,
                                tile_position=(bp, 0),
                            )
                        pt = ptp.tile(
                            [128, NQ], bf16, tag="pt", name=f"pt{pair}_{kt}_{hi}"
                        )
                        nc.scalar.activation(
                            pt, st, AF.Exp, bias=dkp_sb[:, kt : kt + 1], scale=1.0
                        )
                        for ns in range(NQ // 512):
                            nc.tensor.matmul(
                                outT[hi][:, ns * 512 : (ns + 1) * 512],
                                v_sb[:, kt, h, :],
                                pt[:, ns * 512 : (ns + 1) * 512],
                                start=(kt == 0),
                                stop=(kt == KT - 1),
                                skip_group_check=True,
                            )

                # ---- tail: broadcast r, 1/r, normalize attn-out into aflat ----
                rrow_f32 = []
                for hi in range(2):
                    rr = rinvp.tile([1, NQ], f32r, tag="rr", name=f"rr{pair}_{hi}")
                    nc.vector.tensor_copy(rr, outT[hi][DH : DH + 1, :])
                    rf = rinvp.tile([1, NQ], f32, tag="rf", name=f"rf{pair}_{hi}")
                    nc.vector.tensor_copy(rf, outT[hi][DH : DH + 1, :])
                    rrow_f32.append(rf)
                    bc = psp.tile([128, NQ], f32, tag="a", name=f"rbc{pair}_{hi}")
                    for ns in range(NQ // 512):
                        nc.tensor.matmul(
                            bc[:, ns * 512 : (ns + 1) * 512],
                            ones_col,
                            rr[:, ns * 512 : (ns + 1) * 512],
                            start=True,
                            stop=True,
                        )
                    rbc = otp.tile([128, NQ], f32, tag="rbc", name=f"rbcs{pair}_{hi}")
                    nc.vector.reciprocal(rbc, bc)
                    ot = otp.tile([DH, NQ], f32, tag="otsb", name=f"ot{pair}_{hi}")
                    nc.vector.tensor_copy(ot, outT[hi][0:DH, :])
                    nc.vector.tensor_mul(
                        aflat[hi * 64 : hi * 64 + DH, pair, :], ot, rbc[0:DH, :]
                    )

                # ---- pass 2: normalized attn matrix out ----
                for hi in range(2):
                    h = pair * 2 + hi
                    rp = psp.tile([128, QT], f32, tag="a", name=f"rp{pair}_{hi}")
                    for qt in range(QT):
                        nc.tensor.matmul(
                            rp[:, qt : qt + 1],
                            rrow_f32[hi][:, qt * 128 : (qt + 1) * 128],
                            ones_f1,
                            start=True,
                            stop=True,
                        )
                    lnr = smallp.tile([128, QT], f32, tag="lnr", name=f"lnr{pair}_{hi}")
                    nc.scalar.activation(lnr, rp, AF.Ln)
                    lnri = smallp.tile(
                        [128, QT], f32, tag="lnri", name=f"lnri{pair}_{hi}"
                    )
                    nc.vector.tensor_scalar_mul(lnri, lnr, -1.0)
                    for qt in range(QT):
                        at = aoutp.tile(
                            [128, N], f32, tag="at", name=f"at{pair}_{hi}_{qt}"
                        )
                        for sh, stag in ((0, "a"), (1, "b")):
                            s = psp.tile(
                                [128, NQ], f32, tag=stag,
                                name=f"s{pair}_{hi}_{qt}_{sh}",
                            )
                            for ns in range(NQ // 512):
                                nc.tensor.matmul(
                                    s[:, ns * 512 : (ns + 1) * 512],
                                    qaug[:, h, qt * 128 : (qt + 1) * 128],
                                    kaug[
                                        :, h, sh * NQ + ns * 512 : sh * NQ
                                        + (ns + 1) * 512
                                    ],
                                    start=True,
                                    stop=True,
                                )
                            nc.scalar.activation(
                                at[:, sh * NQ : (sh + 1) * NQ],
                                s,
                                AF.Exp,
                                bias=lnri[:, qt : qt + 1],
                                scale=1.0,
                            )
                        nc.gpsimd.dma_start(
                            out=attn_d[h, qt * 128 : (qt + 1) * 128, :], in_=at
                        )

        # ---------- unify + layernorm ----------
        for rt in range(QT):
            psu = psp.tile([128, D], f32, tag="b", name=f"psu{rt}")
            for ic in range(4):
                nc.tensor.matmul(
                    psu,
                    aflat[:, ic, rt * 128 : (rt + 1) * 128],
                    wu_sb[:, ic, :],
                    start=(ic == 0),
                    stop=(ic == 3),
                )
            tmp = yp.tile([128, D], f32, tag="tmp", name=f"tmp{rt}")
            nc.vector.tensor_add(tmp, psu, bu_bc)
            stats = smallp.tile([128, 6], f32, tag="stats", name=f"stats{rt}")
            nc.vector.bn_stats(stats, tmp)
            mv = smallp.tile([128, 2], f32, tag="mv", name=f"mv{rt}")
            nc.vector.bn_aggr(mv, stats)
            sq = smallp.tile([128, 1], f32, tag="sq", name=f"sq{rt}")
            nc.scalar.activation(sq, mv[:, 1:2], AF.Sqrt, bias=eps_sb)
            rstd = smallp.tile([128, 1], f32, tag="rstd", name=f"rstd{rt}")
            nc.vector.reciprocal(rstd, sq)
            xn = yp.tile([128, D], f32, tag="xn", name=f"xn{rt}")
            nc.vector.tensor_scalar(
                out=xn,
                in0=tmp,
                scalar1=mv[:, 0:1],
                scalar2=rstd,
                op0=OP.subtract,
                op1=OP.mult,
            )
            nc.vector.tensor_mul(xn, xn, gam_bc)
            nc.vector.tensor_add(xn, xn, bet_bc)
            nc.gpsimd.dma_start(out=y_d[rt * 128 : (rt + 1) * 128, :], in_=xn)

    nc.compile()
    return nc


def _make_in_maps(x, w_qkv, w_unify, b_unify, ln_gamma, ln_beta):
    import ml_dtypes

    bf = ml_dtypes.bfloat16
    wq = np.array(w_qkv, dtype=np.float32, copy=True)
    wq[:, :INNER] *= DH ** -0.5
    wq_bf = np.ascontiguousarray(wq.astype(bf))
    wu = np.ascontiguousarray(np.asarray(w_unify, dtype=np.float32))
    decay = np.exp(-np.arange(N, dtype=np.float64) / DECAY_TAU).astype(np.float32)
    decay_pc = np.ascontiguousarray(decay.reshape(KT, 128).T)
    decay_bf = np.ascontiguousarray(decay.reshape(1, N).astype(bf))
    bu = np.ascontiguousarray(np.asarray(b_unify, np.float32).reshape(1, D))
    gam = np.ascontiguousarray(np.asarray(ln_gamma, np.float32).reshape(1, D))
    bet = np.ascontiguousarray(np.asarray(ln_beta, np.float32).reshape(1, D))
    ones_row = np.ones((1, NQ), np.float32)
    ones_bf = np.ones((1, NQ), bf)

    in_maps = []
    for c in range(NCORES):
        b_i, qh = c // 2, c % 2
        xT = np.ascontiguousarray(np.asarray(x[b_i], np.float32).T.astype(bf))
        xTq = np.ascontiguousarray(xT[:, qh * NQ : (qh + 1) * NQ])
        in_maps.append(
            {
                "xT": xT,
                "xTq": xTq,
                "wqkv": wq_bf,
                "wu": wu,
                "bu": bu,
                "gamma": gam,
                "beta": bet,
                "decay_pc": decay_pc,
                "decay_bf": decay_bf,
                "ones_row": ones_row,
                "ones_bf": ones_bf,
            }
        )
    return in_maps


def kernel(x, w_qkv, w_unify, b_unify, ln_gamma, ln_beta, _collect_results=None):
    from concourse.bass_utils import run_bass_kernel_spmd

    global _PROG
    if _PROG is None:
        _PROG = _build_program()
    nc = _PROG

    in_maps = _make_in_maps(x, w_qkv, w_unify, b_unify, ln_gamma, ln_beta)
    res = run_bass_kernel_spmd(nc, in_maps, list(range(NCORES)))
    if _collect_results is not None:
        _collect_results.append(res)
    outs = res.results

    y = np.empty((B, N, D), np.float32)
    attn = np.empty((B, H, N, N), np.float32)
    for c in range(NCORES):
        b_i, qh = c // 2, c % 2
        sl = slice(qh * NQ, (qh + 1) * NQ)
        y[b_i, sl] = outs[c]["y"]
        attn[b_i, :, sl] = outs[c]["attn"]
    return y, attn


if __name__ == "__main__":
    nc = _build_program()
    print("program built OK")
